# revision 1
# baseline (speedup 1.0000x reference)
"""Bass/Tile kernel for one transformer decoder layer, data-parallel over batch.

Per-core work (one batch element): LN1 -> QKV -> causal attention with
query-axis softmax -> LN2 -> FFN(gelu).

Key math note: the reference softmaxes over the QUERY axis (axis=2), i.e. each
key-column k is normalized over queries q >= k. Therefore
  out[q,d] = sum_k exp(s[q,k]) * (V[k,d] / Z[k]),   Z[k] = sum_{q>=k} exp(s[q,k])
so the 1/Z folds into V's rows and no [T,T] division is needed.
We compute ST = S^T in [k, q] layout (ST = K @ Q^T / sqrt(E)) so that
Z is a free-axis reduction and the AV matmul needs no transposes.

Structure per core:
  x [T,E] --LN1--> xn --PE transpose--> xnT [E,T] fp32r (g/b fused on evict)
  V = xn @ Wv (token-major), then per head pair tt:
      QT/KT rows for pair tt (rotating slots)
      ST blocks -> additive causal mask on PSUM -> one wide exp -> E1T
      previous pair: Z = rowsum(E1T), V' = V * (1/Z), O += E1T^T @ V'
      (O accumulates straight into a [q, (qi,h,d)] concat tile)
  LN2(concat) -> anT;  FFN in two ff-halves (weights fetched once),
  gelu+bias fused;  out written transposed [E,T] (host transposes back).
All big matmuls run in fp32r (fp32 storage, full-rate PE mode); attention
probabilities/V are bf16.
"""

from contextlib import ExitStack

import numpy as np

import concourse.bass as bass
import concourse.tile as tile
import concourse.bacc as bacc
from concourse import mybir

F32 = mybir.dt.float32
F32R = mybir.dt.float32r
BF16 = mybir.dt.bfloat16
AF = mybir.ActivationFunctionType
ALU = mybir.AluOpType
AX = mybir.AxisListType

P = 128
T = 1024
E = 1024
H = 16
HD = 64
FF = 4096
NT = T // P
NE = E // P
NF = FF // P
SCALE = 1.0 / 32.0  # 1/sqrt(E)
EPS = 1e-5

ATT_DT = BF16

# packed const layout (columns in the "consts" dram tensor)
_CONST_COLS = {
    "bqt": (0, NE), "bkt": (NE, NE), "b2t": (2 * NE, NE), "g1t": (3 * NE, NE),
    "be1t": (4 * NE, NE), "g2t": (5 * NE, NE), "be2t": (6 * NE, NE),
    "b1t": (7 * NE, NF), "ident": (7 * NE + NF, P), "maskd": (7 * NE + NF + P, P),
    "bvb": (7 * NE + NF + 2 * P, E),
}
_CONST_W = 7 * NE + NF + 2 * P + E


def build_decoder(debug=False, att_dt=None, reps=1):
    global ATT_DT
    if att_dt is not None:
        ATT_DT = att_dt
    nc = bacc.Bacc(None, target_bir_lowering=False, debug=debug)

    # ---------------- DRAM I/O ----------------
    x_d = nc.dram_tensor("x", (T, E), F32, kind="ExternalInput")
    wq_d = nc.dram_tensor("wq", (NE, P, E), F32R, kind="ExternalInput")
    wk_d = nc.dram_tensor("wk", (NE, P, E), F32R, kind="ExternalInput")
    wv_d = nc.dram_tensor("wv", (E, E), F32R, kind="ExternalInput")
    w1_d = nc.dram_tensor("w1r", (NF, P, E), F32R, kind="ExternalInput")
    w2_d = nc.dram_tensor("w2r", (NE, P, FF), F32R, kind="ExternalInput")
    c_d = nc.dram_tensor("consts", (P, _CONST_W), F32, kind="ExternalInput")
    y_d = nc.dram_tensor("yT", (E, T), F32, kind="ExternalOutput")

    with ExitStack() as es:
        tc = es.enter_context(tile.TileContext(nc))

        const = es.enter_context(tc.tile_pool(name="const", bufs=1, side="left"))
        # one PSUM pool, 8 banks total: 3x [P,1024] (proj/scores/FFN) +
        # 2x [P,512] shared by AV accumulation and LN transposes (never overlap)
        psq = es.enter_context(tc.tile_pool(name="psq", bufs=1, space="PSUM"))

        cz = const.tile([P, _CONST_W], F32)
        nc.sync.dma_start(cz, c_d[:, :])

        def cview(name):
            o, w = _CONST_COLS[name]
            return cz[:, o:o + w]

        bq_t, bk_t, b2_t = cview("bqt"), cview("bkt"), cview("b2t")
        g1_t, be1_t, g2_t, be2_t = (cview("g1t"), cview("be1t"),
                                    cview("g2t"), cview("be2t"))
        b1_t, ident, maskd, bv_b = (cview("b1t"), cview("ident"),
                                    cview("maskd"), cview("bvb"))
        eps_t = const.tile([P, 1], F32)
        nc.vector.memset(eps_t, EPS)
        ident_r = const.tile([P, P], F32R)
        nc.vector.tensor_copy(ident_r, ident)

        def sc_tile(name):
            return psq.tile([P, 2 * 512], F32, tag="sc", bufs=3, name=name)

        def av_tile(name, dt=F32):
            return psq.tile([P, 512], dt, tag="avtr", bufs=2, name=name)

        def layernorm_to_transposed(src_fn, dst_tiles, g_t, b_t, ln_pool):
            """src_fn(ti) -> [P, E] token-major AP.  Writes
            dst_tiles[j][:, ti*P:(ti+1)*P] = norm(src)^T * g + b (feature-major)."""
            for ti in range(NT):
                xsrc = src_fn(ti)
                stats = ln_pool.tile([P, 2, 6], F32, tag="bnstats")
                for sg in range(2):
                    nc.vector.bn_stats(out=stats[:, sg, :],
                                       in_=xsrc[:, sg * 512:(sg + 1) * 512])
                mv = ln_pool.tile([P, 2], F32, tag="bnmv")
                nc.vector.bn_aggr(out=mv, in_=stats)
                nc.scalar.activation(out=mv[:, 1:2], in_=mv[:, 1:2], func=AF.Sqrt,
                                     bias=eps_t)
                nc.vector.reciprocal(mv[:, 1:2], mv[:, 1:2])
                xn = ln_pool.tile([P, E], F32R, tag="xn")
                nc.vector.tensor_scalar(
                    out=xn, in0=xsrc, scalar1=mv[:, 0:1], scalar2=mv[:, 1:2],
                    op0=ALU.subtract, op1=ALU.mult)
                for j in range(NE):
                    ptr = av_tile(f"tr{ti}_{j}", dt=F32R)[:, :P]
                    nc.tensor.transpose(ptr, xn[:, j * P:(j + 1) * P], ident_r)
                    nc.scalar.activation(
                        out=dst_tiles[j][:, ti * P:(ti + 1) * P], in_=ptr,
                        func=AF.Identity, bias=b_t[:, j:j + 1], scale=g_t[:, j:j + 1])

        rep_cm = tc.For_i(0, reps, 1) if reps > 1 else None
        if rep_cm is not None:
            rep_cm.__enter__()

        # =========== Phase 1: LN1 + transpose ===========
        xnT_pool = tc.alloc_tile_pool(name="xnT", bufs=1, side="right")
        xnT = [xnT_pool.tile([P, T], F32R, tag=f"xnT{j}", name=f"xnT{j}")
               for j in range(NE)]
        with tc.tile_pool(name="ph1", bufs=3, side="left") as ph1, \
             tc.tile_pool(name="xin", bufs=NT, side="left") as xin:
            x_tiles = []
            for ti in range(NT):
                xt = xin.tile([P, E], F32, tag="x")
                nc.sync.dma_start(xt, x_d[ti * P:(ti + 1) * P, :])
                x_tiles.append(xt)
            layernorm_to_transposed(lambda ti: x_tiles[ti][:, :], xnT,
                                    g1_t, be1_t, ph1)

        # ====== Phase 2+3: V, then per-pair {Q/K proj -> scores+exp -> AV} ======
        concat_pool = tc.alloc_tile_pool(name="concat", bufs=1, side="left")
        concat = concat_pool.tile([P, NT * E], ATT_DT)
        concat_v = concat.rearrange("p (a h d) -> p a h d", a=NT, h=H)

        vt_pool = tc.alloc_tile_pool(name="vtp", bufs=1, side="left")
        vt = [vt_pool.tile([P, E], ATT_DT, tag=f"vt{i}", name=f"vt{i}")
              for i in range(NT)]

        # --- V projection (token-major) ---
        with tc.tile_pool(name="wv", bufs=NE + 1, side="left") as wvp:
            wtl = []
            for ko in range(NE):
                wtile = wvp.tile([P, E], F32R, tag="w", name=f"wv{ko}")
                nc.sync.dma_start(wtile, wv_d[ko * P:(ko + 1) * P, :])
                wtl.append(wtile)
            for ti in range(NT):
                ps = sc_tile(f"psv{ti}")
                for nh in range(2):
                    for ko in range(NE):
                        nc.tensor.matmul(
                            ps[:, nh * 512:(nh + 1) * 512],
                            lhsT=xnT[ko][:, ti * P:(ti + 1) * P],
                            rhs=wtl[ko][:, nh * 512:(nh + 1) * 512],
                            start=(ko == 0), stop=(ko == NE - 1))
                nc.vector.tensor_add(out=ps, in0=ps, in1=bv_b)
                nc.vector.tensor_copy(out=vt[ti], in_=ps)

        # --- interleaved Q/K projection + attention ---
        qk_pool = tc.alloc_tile_pool(name="qk", bufs=3, side="left")
        wqk_pool = tc.alloc_tile_pool(name="wqk", bufs=6, side="left")
        e1p = tc.alloc_tile_pool(name="e1", bufs=32, side="left")
        vpp = tc.alloc_tile_pool(name="vp", bufs=16, side="left")
        zsp = tc.alloc_tile_pool(name="zs", bufs=4, side="left")

        def emit_qk(tt):
            qtt = qk_pool.tile([P, T], F32R, tag="qt", name=f"qt{tt}")
            ktt = qk_pool.tile([P, T], F32R, tag="kt", name=f"kt{tt}")
            for dst, w_d, b_t in ((qtt, wq_d, bq_t), (ktt, wk_d, bk_t)):
                wsl = wqk_pool.tile([P, NE, P], F32R, tag="wqk")
                nc.sync.dma_start(wsl, w_d[tt].rearrange("p (a b) -> p a b", a=NE))
                ps = sc_tile(f"psqk{tt}")
                for th in range(2):
                    for ko in range(NE):
                        nc.tensor.matmul(
                            ps[:, th * 512:(th + 1) * 512],
                            lhsT=wsl[:, ko, :],
                            rhs=xnT[ko][:, th * 512:(th + 1) * 512],
                            start=(ko == 0), stop=(ko == NE - 1))
                nc.vector.tensor_scalar(
                    out=dst, in0=ps, scalar1=b_t[:, tt:tt + 1], scalar2=None,
                    op0=ALU.add)
            return qtt, ktt

        def emit_scores(tt, qtt, ktt, e1s, zhs):
            """ST blocks + additive mask + one wide exp per (h, ki); even/odd
            heads interleaved so their K=64 matmuls share the PE array.
            Z reductions run eagerly after each exp so the AV stage only
            waits on reciprocal+V'."""
            for ki in range(NT):
                for h in (2 * tt, 2 * tt + 1):
                    po = HD * (h % 2)
                    e1ki = e1p.tile([P, T], ATT_DT, tag="e1t", name=f"e1_{h}_{ki}")
                    e1s[(h, ki)] = e1ki
                    c0 = (ki * P) // 512
                    sps = sc_tile(f"pss{h}_{ki}")
                    for c in range(c0, 2):
                        nc.tensor.matmul(
                            sps[:, c * 512:(c + 1) * 512],
                            lhsT=ktt[po:po + HD, ki * P:(ki + 1) * P],
                            rhs=qtt[po:po + HD, c * 512:(c + 1) * 512],
                            start=True, stop=True)
                    nc.vector.tensor_add(
                        out=sps[:, ki * P:(ki + 1) * P],
                        in0=sps[:, ki * P:(ki + 1) * P], in1=maskd)
                    nc.scalar.activation(
                        out=e1ki[:, ki * P:], in_=sps[:, ki * P:],
                        func=AF.Exp, scale=SCALE)

        def emit_av(hp, e1s, zhs):
            for h in (2 * hp, 2 * hp + 1):
                zh = zsp.tile([P, NT], F32, tag="zh")
                for ki in range(NT):
                    nc.vector.reduce_sum(out=zh[:, ki:ki + 1],
                                         in_=e1s[(h, ki)][:, ki * P:], axis=AX.X)
                rz = zsp.tile([P, NT], F32, tag="rz")
                nc.vector.reciprocal(rz, zh)
                vps = []
                for ki in range(NT):
                    vp_t = vpp.tile([P, HD], ATT_DT, tag="vp")
                    nc.vector.tensor_scalar_mul(
                        vp_t, vt[ki][:, h * HD:(h + 1) * HD], rz[:, ki:ki + 1])
                    vps.append(vp_t)
                po_ps = av_tile(f"psav{h}")
                for qi in range(NT):
                    for ki in range(qi + 1):
                        nc.tensor.matmul(
                            po_ps[:, qi * HD:(qi + 1) * HD],
                            lhsT=e1s[(h, ki)][:, qi * P:(qi + 1) * P],
                            rhs=vps[ki],
                            start=(ki == 0), stop=(ki == qi))
                nc.vector.tensor_copy(
                    out=concat_v[:, :, h, :],
                    in_=po_ps.rearrange("p (a d) -> p a d", a=NT))

        e1s_by_hp = {}
        zhs = {}
        for tt in range(H // 2):
            qtt, ktt = emit_qk(tt)
            e1s_by_hp[tt] = {}
            emit_scores(tt, qtt, ktt, e1s_by_hp[tt], zhs)
            if tt >= 1:
                emit_av(tt - 1, e1s_by_hp.pop(tt - 1), zhs)
        emit_av(H // 2 - 1, e1s_by_hp.pop(H // 2 - 1), zhs)

        zsp.release(); vpp.release(); e1p.release()
        wqk_pool.release(); qk_pool.release()
        vt_pool.release()
        xnT_pool.release()

        # =========== Phase 4: LN2 + transpose ===========
        anT_pool = tc.alloc_tile_pool(name="anT", bufs=1, side="right")
        anT = [anT_pool.tile([P, T], F32R, tag=f"anT{j}", name=f"anT{j}")
               for j in range(NE)]
        with tc.tile_pool(name="ph4", bufs=3, side="left") as ph4:
            layernorm_to_transposed(
                lambda ti: concat[:, ti * E:(ti + 1) * E], anT, g2_t, be2_t, ph4)
        concat_pool.release()

        # =========== Phase 5: FFN (two ff halves; weights fetched once) ===========
        NFH = NF // 2
        out_pool = tc.alloc_tile_pool(name="outT", bufs=1, side="right")
        outT = [out_pool.tile([P, T], F32, tag=f"o{j}", name=f"o{j}")
                for j in range(NE)]
        with tc.tile_pool(name="w1s", bufs=4, side="left") as w1s, \
             tc.tile_pool(name="w2s", bufs=3, side="left") as w2s, \
             tc.tile_pool(name="hid", bufs=NF // 2 + 1, side="left") as hidp:
            for ffh in range(2):
                hid = []
                for fl in range(NFH):
                    fo = ffh * NFH + fl
                    w1t = w1s.tile([P, NE, P], F32R, tag="w1")
                    nc.sync.dma_start(w1t, w1_d[fo].rearrange("p (a b) -> p a b", a=NE))
                    hid_fo = hidp.tile([P, T], F32R, tag="hid", name=f"hid{fo}")
                    ps = sc_tile(f"psf{fo}")
                    for th in range(2):
                        for ko in range(NE):
                            nc.tensor.matmul(
                                ps[:, th * 512:(th + 1) * 512],
                                lhsT=w1t[:, ko, :],
                                rhs=anT[ko][:, th * 512:(th + 1) * 512],
                                start=(ko == 0), stop=(ko == NE - 1))
                    nc.scalar.activation(
                        out=hid_fo, in_=ps, func=AF.Gelu, bias=b1_t[:, fo:fo + 1])
                    hid.append(hid_fo)
                for eo in range(NE):
                    w2t = w2s.tile([P, NFH, P], F32R, tag="w2")
                    nc.sync.dma_start(
                        w2t, w2_d[eo, :, ffh * NFH * P:(ffh + 1) * NFH * P]
                        .rearrange("p (a b) -> p a b", a=NFH))
                    ps = sc_tile(f"pso{ffh}_{eo}")
                    for th in range(2):
                        for kl in range(NFH):
                            nc.tensor.matmul(
                                ps[:, th * 512:(th + 1) * 512],
                                lhsT=w2t[:, kl, :],
                                rhs=hid[kl][:, th * 512:(th + 1) * 512],
                                start=(kl == 0), stop=(kl == NFH - 1))
                    if ffh == 0:
                        nc.scalar.activation(
                            out=outT[eo], in_=ps,
                            func=AF.Identity, bias=b2_t[:, eo:eo + 1])
                    else:
                        nc.vector.tensor_add(out=outT[eo], in0=ps, in1=outT[eo])
        for eo in range(NE):
            nc.sync.dma_start(y_d[eo * P:(eo + 1) * P, :], outT[eo])
        out_pool.release()
        anT_pool.release()
        if rep_cm is not None:
            rep_cm.__exit__(None, None, None)

    nc.compile()
    return nc


def host_inputs(core_x, Wq, bq, Wk, bk, Wv, bv, W1, b1, W2, b2, g1, be1, g2, be2):
    """Build the per-core in_map from full weights + this core's x shard [T, E]."""
    f = np.float32

    consts = np.zeros((P, _CONST_W), f)

    def put(name, arr):
        o, w = _CONST_COLS[name]
        consts[:, o:o + w] = arr

    put("bqt", np.asarray(bq, f).reshape(NE, P).T)
    put("bkt", np.asarray(bk, f).reshape(NE, P).T)
    put("b2t", np.asarray(b2, f).reshape(NE, P).T)
    put("g1t", np.asarray(g1, f).reshape(NE, P).T)
    put("be1t", np.asarray(be1, f).reshape(NE, P).T)
    put("g2t", np.asarray(g2, f).reshape(NE, P).T)
    put("be2t", np.asarray(be2, f).reshape(NE, P).T)
    put("b1t", np.asarray(b1, f).reshape(NF, P).T)
    put("ident", np.eye(P, dtype=f))
    put("maskd", np.where(np.triu(np.ones((P, P), bool)), 0.0, -6000.0).astype(f))
    put("bvb", np.broadcast_to(np.asarray(bv, f), (P, E)))

    return {
        "x": np.ascontiguousarray(core_x, f),
        "wq": np.ascontiguousarray(
            np.asarray(Wq, f).reshape(NE, P, NE, P).transpose(2, 1, 0, 3)
            .reshape(NE, P, E)),
        "wk": np.ascontiguousarray(
            np.asarray(Wk, f).reshape(NE, P, NE, P).transpose(2, 1, 0, 3)
            .reshape(NE, P, E)),
        "wv": np.ascontiguousarray(Wv, f),
        "w1r": np.ascontiguousarray(
            np.asarray(W1, f).reshape(NE, P, NF, P).transpose(2, 1, 0, 3)
            .reshape(NF, P, E)),
        "w2r": np.ascontiguousarray(
            np.asarray(W2, f).reshape(NF, P, NE, P).transpose(2, 1, 0, 3)
            .reshape(NE, P, FF)),
        "consts": consts,
    }



# ======================================================================
# Harness entry point: full-input kernel with internal batch sharding
# ======================================================================

_NC_CACHE = {}


def _get_nc():
    key = ("bf16" if ATT_DT == BF16 else "f32")
    if key not in _NC_CACHE:
        _NC_CACHE[key] = build_decoder()
    return _NC_CACHE[key]


def kernel(x, Wq, bq, Wk, bk, Wv, bv, W1, b1, W2, b2, g1, be1, g2, be2):
    """Full-input entry: x [8, 1024, 1024]; returns [8, 1024, 1024] float32.

    Shards batch across the 8 NeuronCores (one element per core), runs the
    Bass decoder kernel SPMD, and gathers/transposes the per-core outputs.
    """
    from concourse.bass_utils import run_bass_kernel_spmd

    x = np.asarray(x, np.float32)
    B = x.shape[0]
    nc = _get_nc()
    args = tuple(np.asarray(a, np.float32) for a in
                 (Wq, bq, Wk, bk, Wv, bv, W1, b1, W2, b2, g1, be1, g2, be2))
    in_maps = [host_inputs(x[c], *args) for c in range(B)]
    res = run_bass_kernel_spmd(nc, in_maps, core_ids=list(range(B)))
    out = np.stack([np.asarray(r["yT"]).T for r in res.results], axis=0)
    return np.ascontiguousarray(out, np.float32)



# revision 5
# speedup vs baseline: 1.0682x; 1.0682x over previous
"""Bass/Tile kernel for one transformer decoder layer, data-parallel over batch.

Per-core work (one batch element): LN1 -> QKV -> causal attention with
query-axis softmax -> LN2 -> FFN(gelu).

Math note: the reference softmaxes over the QUERY axis, i.e. each key-column k
is normalized over queries q >= k:
  out[q,d] = sum_k exp(s[q,k]) * (V[k,d] / Z[k]),   Z[k] = sum_{q>=k} exp(s[q,k])
We compute ST = S^T in [k, q] layout (ST = K @ Q^T / sqrt(E)) so Z is a
free-axis reduction and the AV matmul needs no transposes.

v2 design (cost-model-driven):
  - all matmul operands bf16 (weights pre-cast on host; 16/32-bit mixing is
    not supported by the PE): full rate at any moving width N.
  - LN affine (g,b) folded into the following projection weights on host, so
    device LN is just (x-mu)*rstd and the transpose evict is a plain copy.
  - causal truncation of score matmuls at 128-col granularity; the causal
    diag-block mask is applied by an extra matmul (ident.T @ maskd) appended
    to the same PSUM accumulation group -- no vector-engine mask pass.
  - Z comes for free from the exp via activation accum_out.
  - FFN2 accumulates all 32 f-blocks in one PSUM group (both ff halves'
    hid tiles are kept resident); FFN1 streams 256-wide quarters so it can
    start while LN2 is still finishing.
  - weight DMAs issued from the (otherwise idle) GPSIMD queue; x/consts/out
    on the sync queue.
  - evictions split between Act and DVE to balance engine load.
"""

from contextlib import ExitStack

import numpy as np

import concourse.bass as bass
import concourse.tile as tile
import concourse.bacc as bacc
from concourse import mybir

F32 = mybir.dt.float32
BF16 = mybir.dt.bfloat16
AF = mybir.ActivationFunctionType
ALU = mybir.AluOpType

P = 128
T = 1024
E = 1024
H = 16
HD = 64
FF = 4096
NT = T // P
NE = E // P
NF = FF // P
SCALE = 1.0 / 32.0  # 1/sqrt(E)
EPS = 1e-5

# packed f32 const columns
_CONST_COLS = {
    "bqt": (0, NE), "bkt": (NE, NE), "b2t": (2 * NE, NE), "b1t": (3 * NE, NF),
}
_CONST_W = 3 * NE + NF
# packed bf16 const columns
_CONSTB_COLS = {"ident": (0, P), "maskd": (P, P), "bvb": (2 * P, E)}
_CONSTB_W = 2 * P + E


def build_decoder(debug=False, reps=1):
    nc = bacc.Bacc(None, target_bir_lowering=False, debug=debug)

    # ---------------- DRAM I/O ----------------
    x_d = nc.dram_tensor("x", (T, E), F32, kind="ExternalInput")
    wq_d = nc.dram_tensor("wq", (NE, P, E), BF16, kind="ExternalInput")
    wk_d = nc.dram_tensor("wk", (NE, P, E), BF16, kind="ExternalInput")
    wv_d = nc.dram_tensor("wv", (E, E), BF16, kind="ExternalInput")
    w1_d = nc.dram_tensor("w1r", (NF, P, E), BF16, kind="ExternalInput")
    w2_d = nc.dram_tensor("w2r", (NE, P, FF), BF16, kind="ExternalInput")
    c_d = nc.dram_tensor("consts", (P, _CONST_W), F32, kind="ExternalInput")
    cb_d = nc.dram_tensor("constsb", (P, _CONSTB_W), BF16, kind="ExternalInput")
    y_d = nc.dram_tensor("yT", (E, T), BF16, kind="ExternalOutput")

    with ExitStack() as es:
        tc = es.enter_context(tile.TileContext(nc))

        const = es.enter_context(tc.tile_pool(name="const", bufs=1, side="left"))
        psq = es.enter_context(tc.tile_pool(name="psq", bufs=1, space="PSUM"))

        cz = const.tile([P, _CONST_W], F32)
        nc.sync.dma_start(cz, c_d[:, :])
        czb = const.tile([P, _CONSTB_W], BF16)
        nc.sync.dma_start(czb, cb_d[:, :])

        def cview(name):
            o, w = _CONST_COLS[name]
            return cz[:, o:o + w]

        def cbview(name):
            o, w = _CONSTB_COLS[name]
            return czb[:, o:o + w]

        bq_t, bk_t, b2_t, b1_t = (cview("bqt"), cview("bkt"), cview("b2t"),
                                  cview("b1t"))
        identb, maskd, bv_b = cbview("ident"), cbview("maskd"), cbview("bvb")
        eps_t = const.tile([P, 1], F32)
        nc.vector.memset(eps_t, EPS)

        def sc_tile(name):
            return psq.tile([P, 2 * 512], F32, tag="sc", bufs=3, name=name)

        def av_tile(name, dt=F32):
            return psq.tile([P, 512], dt, tag="avtr", bufs=2, name=name)

        rep_cm = tc.For_i(0, reps, 1) if reps > 1 else None
        if rep_cm is not None:
            rep_cm.__enter__()

        # =========== Phase 1: x DMA + LN1 + transpose + V proj, per tile =====
        xnT_pool = tc.alloc_tile_pool(name="xnT", bufs=1, side="right")
        xnT = [xnT_pool.tile([P, T], BF16, tag=f"xnT{j}", name=f"xnT{j}")
               for j in range(NE)]
        concat_pool = tc.alloc_tile_pool(name="concat", bufs=1, side="left")
        concat = concat_pool.tile([P, NT * E], BF16)
        concat_v = concat.rearrange("p (a h d) -> p a h d", a=NT, h=H)
        vt_pool = tc.alloc_tile_pool(name="vtp", bufs=1, side="left")
        vt = [vt_pool.tile([P, E], BF16, tag=f"vt{i}", name=f"vt{i}")
              for i in range(NT)]

        def ln_tile(src_fn, dst_tiles, ti, ln_pool, evict_sel):
            """LN (no affine) of one [P, E] token tile + transpose into
            dst_tiles[j][:, ti*P:(ti+1)*P]."""
            stats = ln_pool.tile([P, 2, 6], F32, tag="bnstats")
            for sg in range(2):
                nc.vector.bn_stats(out=stats[:, sg, :],
                                   in_=src_fn(ti)[:, sg * 512:(sg + 1) * 512])
            mv = ln_pool.tile([P, 2], F32, tag="bnmv")
            nc.vector.bn_aggr(out=mv, in_=stats)
            nc.scalar.activation(out=mv[:, 1:2], in_=mv[:, 1:2], func=AF.Sqrt,
                                 bias=eps_t)
            nc.vector.reciprocal(mv[:, 1:2], mv[:, 1:2])
            xn = ln_pool.tile([P, E], BF16, tag="xn")
            nc.vector.tensor_scalar(
                out=xn, in0=src_fn(ti), scalar1=mv[:, 0:1], scalar2=mv[:, 1:2],
                op0=ALU.subtract, op1=ALU.mult)
            for j in range(NE):
                ptr = av_tile(f"tr{ti}_{j}", dt=BF16)[:, :P]
                nc.tensor.transpose(ptr, xn[:, j * P:(j + 1) * P], identb)
                dst = dst_tiles[j][:, ti * P:(ti + 1) * P]
                if (ti * NE + j) % 2 == evict_sel:
                    nc.scalar.activation(out=dst, in_=ptr, func=AF.Identity)
                else:
                    nc.vector.tensor_copy(out=dst, in_=ptr)

        with tc.tile_pool(name="ph1", bufs=3, side="left") as ph1, \
             tc.tile_pool(name="xin", bufs=NT, side="left") as xin, \
             tc.tile_pool(name="wv", bufs=NE, side="left") as wvp:
            x_tiles = []
            for ti in range(NT):
                xt = xin.tile([P, E], F32, tag="x")
                for hh in range(2):
                    nc.sync.dma_start(xt[:, hh * 512:(hh + 1) * 512],
                                      x_d[ti * P:(ti + 1) * P,
                                          hh * 512:(hh + 1) * 512])
                x_tiles.append(xt)
            wtl = []
            for ko in range(NE):
                wtile = wvp.tile([P, E], BF16, tag="w", name=f"wv{ko}")
                nc.gpsimd.dma_start(wtile, wv_d[ko * P:(ko + 1) * P, :])
                wtl.append(wtile)
            for ti in range(NT):
                ln_tile(lambda t: x_tiles[t][:, :], xnT, ti, ph1, evict_sel=0)
                # V projection for this token tile (needs xnT[:][ti block])
                ps = sc_tile(f"psv{ti}")
                for nh in range(2):
                    for ko in range(NE):
                        nc.tensor.matmul(
                            ps[:, nh * 512:(nh + 1) * 512],
                            lhsT=xnT[ko][:, ti * P:(ti + 1) * P],
                            rhs=wtl[ko][:, nh * 512:(nh + 1) * 512],
                            start=(ko == 0), stop=(ko == NE - 1))
                nc.vector.tensor_tensor(out=vt[ti], in0=ps, in1=bv_b,
                                        op=ALU.add)

        # ====== Phase 2: per-pair {Q/K proj -> scores+exp(+Z) -> AV} ======
        qk_pool = tc.alloc_tile_pool(name="qk", bufs=3, side="left")
        wqk_pool = tc.alloc_tile_pool(name="wqk", bufs=6, side="left")
        e1p = tc.alloc_tile_pool(name="e1", bufs=32, side="left")
        vpp = tc.alloc_tile_pool(name="vp", bufs=16, side="left")
        zsp = tc.alloc_tile_pool(name="zs", bufs=4, side="left")

        def emit_qk(tt):
            qtt = qk_pool.tile([P, T], BF16, tag="qt", name=f"qt{tt}")
            ktt = qk_pool.tile([P, T], BF16, tag="kt", name=f"kt{tt}")
            for dst, w_d, b_t in ((qtt, wq_d, bq_t), (ktt, wk_d, bk_t)):
                wsl = wqk_pool.tile([P, NE, P], BF16, tag="wqk")
                nc.gpsimd.dma_start(
                    wsl, w_d[tt].rearrange("p (a b) -> p a b", a=NE))
                ps = sc_tile(f"psqk{tt}")
                for th in range(2):
                    for ko in range(NE):
                        nc.tensor.matmul(
                            ps[:, th * 512:(th + 1) * 512],
                            lhsT=wsl[:, ko, :],
                            rhs=xnT[ko][:, th * 512:(th + 1) * 512],
                            start=(ko == 0), stop=(ko == NE - 1))
                nc.vector.tensor_scalar(
                    out=dst, in0=ps, scalar1=b_t[:, tt:tt + 1], scalar2=None,
                    op0=ALU.add)
            return qtt, ktt

        def emit_scores(tt, qtt, ktt, e1s, zhs):
            """Causal-truncated ST blocks; diag mask folded into the PSUM
            accumulation group as ident.T @ maskd; one wide exp per (h, ki)
            with Z accumulated for free."""
            for h in (2 * tt, 2 * tt + 1):
                zh = zsp.tile([P, NT], F32, tag="zh", name=f"zh{h}")
                zhs[h] = zh
            for ki in range(NT):
                qs = ki * P
                # chunk [qs..T) on 512 boundaries
                chunks = []
                c0 = qs
                if c0 % 512:
                    nxt = (c0 // 512 + 1) * 512
                    chunks.append((c0, nxt))
                    c0 = nxt
                while c0 < T:
                    chunks.append((c0, c0 + 512))
                    c0 += 512
                for h in (2 * tt, 2 * tt + 1):
                    po = HD * (h % 2)
                    e1ki = e1p.tile([P, T], BF16, tag="e1t", name=f"e1_{h}_{ki}")
                    e1s[(h, ki)] = e1ki
                    sps = sc_tile(f"pss{h}_{ki}")
                    for (a, b) in chunks:
                        has_diag = a == qs
                        nc.tensor.matmul(
                            sps[:, a:b],
                            lhsT=ktt[po:po + HD, ki * P:(ki + 1) * P],
                            rhs=qtt[po:po + HD, a:b],
                            start=True, stop=not has_diag)
                        if has_diag:
                            nc.tensor.matmul(
                                sps[:, qs:qs + P],
                                lhsT=identb, rhs=maskd,
                                start=False, stop=True, skip_group_check=True)
                    nc.scalar.activation(
                        out=e1ki[:, qs:], in_=sps[:, qs:],
                        func=AF.Exp, scale=SCALE,
                        accum_out=zhs[h][:, ki:ki + 1])

        def emit_av(hp, e1s, zhs):
            for h in (2 * hp, 2 * hp + 1):
                rz = zsp.tile([P, NT], F32, tag="rz")
                nc.vector.reciprocal(rz, zhs[h])
                vps = []
                for ki in range(NT):
                    vp_t = vpp.tile([P, HD], BF16, tag="vp")
                    nc.vector.tensor_scalar_mul(
                        vp_t, vt[ki][:, h * HD:(h + 1) * HD], rz[:, ki:ki + 1])
                    vps.append(vp_t)
                po_ps = av_tile(f"psav{h}")
                for qi in range(NT):
                    for ki in range(qi + 1):
                        nc.tensor.matmul(
                            po_ps[:, qi * HD:(qi + 1) * HD],
                            lhsT=e1s[(h, ki)][:, qi * P:(qi + 1) * P],
                            rhs=vps[ki],
                            start=(ki == 0), stop=(ki == qi))
                nc.vector.tensor_copy(
                    out=concat_v[:, :, h, :],
                    in_=po_ps.rearrange("p (a d) -> p a d", a=NT))

        e1s_by_hp = {}
        zhs = {}
        for tt in range(H // 2):
            qtt, ktt = emit_qk(tt)
            e1s_by_hp[tt] = {}
            emit_scores(tt, qtt, ktt, e1s_by_hp[tt], zhs)
            if tt >= 1:
                emit_av(tt - 1, e1s_by_hp.pop(tt - 1), zhs)
        emit_av(H // 2 - 1, e1s_by_hp.pop(H // 2 - 1), zhs)

        zsp.release(); vpp.release(); e1p.release()
        wqk_pool.release(); qk_pool.release()
        vt_pool.release()
        xnT_pool.release()

        # =========== Phase 3: LN2 + transpose ===========
        anT_pool = tc.alloc_tile_pool(name="anT", bufs=1, side="right")
        anT = [anT_pool.tile([P, T], BF16, tag=f"anT{j}", name=f"anT{j}")
               for j in range(NE)]
        with tc.tile_pool(name="ph4", bufs=3, side="left") as ph4:
            for ti in range(NT):
                ln_tile(lambda t: concat[:, t * E:(t + 1) * E], anT, ti, ph4,
                        evict_sel=1)
        concat_pool.release()

        # =========== Phase 4: FFN ===========
        out_pool = tc.alloc_tile_pool(name="outT", bufs=1, side="right")
        outT = [out_pool.tile([P, T], BF16, tag=f"o{j}", name=f"o{j}")
                for j in range(NE)]
        with tc.tile_pool(name="w1s", bufs=4, side="left") as w1s, \
             tc.tile_pool(name="w2s", bufs=2, side="left") as w2s, \
             tc.tile_pool(name="hid", bufs=NF, side="left") as hidp:
            hid = []
            for fo in range(NF):
                w1t = w1s.tile([P, NE, P], BF16, tag="w1")
                nc.gpsimd.dma_start(
                    w1t, w1_d[fo].rearrange("p (a b) -> p a b", a=NE))
                hid_fo = hidp.tile([P, T], BF16, tag="hid", name=f"hid{fo}")
                ps = sc_tile(f"psf{fo}")
                for qt in range(4):
                    for ko in range(NE):
                        nc.tensor.matmul(
                            ps[:, qt * 256:(qt + 1) * 256],
                            lhsT=w1t[:, ko, :],
                            rhs=anT[ko][:, qt * 256:(qt + 1) * 256],
                            start=(ko == 0), stop=(ko == NE - 1))
                    if qt % 2 == 1:
                        th = qt // 2
                        nc.scalar.activation(
                            out=hid_fo[:, th * 512:(th + 1) * 512],
                            in_=ps[:, th * 512:(th + 1) * 512],
                            func=AF.Gelu, bias=b1_t[:, fo:fo + 1])
                hid.append(hid_fo)
            for eo in range(NE):
                w2t = w2s.tile([P, NF, P], BF16, tag="w2")
                nc.gpsimd.dma_start(
                    w2t, w2_d[eo].rearrange("p (a b) -> p a b", a=NF))
                ps = sc_tile(f"pso{eo}")
                for th in range(2):
                    for kl in range(NF):
                        nc.tensor.matmul(
                            ps[:, th * 512:(th + 1) * 512],
                            lhsT=w2t[:, kl, :],
                            rhs=hid[kl][:, th * 512:(th + 1) * 512],
                            start=(kl == 0), stop=(kl == NF - 1))
                nc.vector.tensor_scalar(
                    out=outT[eo], in0=ps, scalar1=b2_t[:, eo:eo + 1],
                    scalar2=None, op0=ALU.add)
                nc.sync.dma_start(y_d[eo * P:(eo + 1) * P, :], outT[eo])
        out_pool.release()
        anT_pool.release()
        if rep_cm is not None:
            rep_cm.__exit__(None, None, None)

    nc.compile()
    return nc


def host_inputs(core_x, Wq, bq, Wk, bk, Wv, bv, W1, b1, W2, b2, g1, be1, g2, be2):
    """Per-core in_map: LN affines folded into the following projections
    (q = ln_raw @ (g1*Wq) + (be1 @ Wq + bq), where ln_raw = (x-mu)*rstd),
    weights pre-cast to bf16 in the block-transposed layouts the kernel
    expects."""
    import ml_dtypes

    f = np.float32
    bf = ml_dtypes.bfloat16

    g1 = np.asarray(g1, f); be1 = np.asarray(be1, f)
    g2 = np.asarray(g2, f); be2 = np.asarray(be2, f)
    Wq = np.asarray(Wq, f); Wk = np.asarray(Wk, f); Wv = np.asarray(Wv, f)
    W1 = np.asarray(W1, f); W2 = np.asarray(W2, f)

    bq_e = (be1 @ Wq + np.asarray(bq, f)).astype(f)
    bk_e = (be1 @ Wk + np.asarray(bk, f)).astype(f)
    bv_e = (be1 @ Wv + np.asarray(bv, f)).astype(f)
    b1_e = (be2 @ W1 + np.asarray(b1, f)).astype(f)

    Wq_s = Wq * g1[:, None]
    Wk_s = Wk * g1[:, None]
    Wv_s = Wv * g1[:, None]
    W1_s = W1 * g2[:, None]

    consts = np.zeros((P, _CONST_W), f)

    def put(name, arr):
        o, w = _CONST_COLS[name]
        consts[:, o:o + w] = arr

    put("bqt", bq_e.reshape(NE, P).T)
    put("bkt", bk_e.reshape(NE, P).T)
    put("b2t", np.asarray(b2, f).reshape(NE, P).T)
    put("b1t", b1_e.reshape(NF, P).T)

    constsb = np.zeros((P, _CONSTB_W), f)

    def putb(name, arr):
        o, w = _CONSTB_COLS[name]
        constsb[:, o:o + w] = arr

    putb("ident", np.eye(P, dtype=f))
    putb("maskd", np.where(np.triu(np.ones((P, P), bool)), 0.0, -6000.0))
    putb("bvb", np.broadcast_to(bv_e, (P, E)))

    return {
        "x": np.ascontiguousarray(core_x, f),
        "wq": np.ascontiguousarray(
            Wq_s.reshape(NE, P, NE, P).transpose(2, 1, 0, 3)
            .reshape(NE, P, E)).astype(bf),
        "wk": np.ascontiguousarray(
            Wk_s.reshape(NE, P, NE, P).transpose(2, 1, 0, 3)
            .reshape(NE, P, E)).astype(bf),
        "wv": np.ascontiguousarray(Wv_s).astype(bf),
        "w1r": np.ascontiguousarray(
            W1_s.reshape(NE, P, NF, P).transpose(2, 1, 0, 3)
            .reshape(NF, P, E)).astype(bf),
        "w2r": np.ascontiguousarray(
            W2.reshape(NF, P, NE, P).transpose(2, 1, 0, 3)
            .reshape(NE, P, FF)).astype(bf),
        "consts": consts,
        "constsb": constsb.astype(bf),
    }


# ======================================================================
# Harness entry point: full-input kernel with internal batch sharding
# ======================================================================

_NC_CACHE = {}


def _get_nc():
    if "nc" not in _NC_CACHE:
        _NC_CACHE["nc"] = build_decoder()
    return _NC_CACHE["nc"]


def kernel(x, Wq, bq, Wk, bk, Wv, bv, W1, b1, W2, b2, g1, be1, g2, be2):
    """Full-input entry: x [8, 1024, 1024]; returns [8, 1024, 1024] float32.

    Shards batch across the 8 NeuronCores (one element per core), runs the
    Bass decoder kernel SPMD, and gathers/transposes the per-core outputs.
    """
    from concourse.bass_utils import run_bass_kernel_spmd

    x = np.asarray(x, np.float32)
    B = x.shape[0]
    nc = _get_nc()
    args = tuple(np.asarray(a, np.float32) for a in
                 (Wq, bq, Wk, bk, Wv, bv, W1, b1, W2, b2, g1, be1, g2, be2))
    in_maps = [host_inputs(x[c], *args) for c in range(B)]
    res = run_bass_kernel_spmd(nc, in_maps, core_ids=list(range(B)))
    out = np.stack([np.asarray(r["yT"], np.float32).T for r in res.results],
                   axis=0)
    return np.ascontiguousarray(out, np.float32)


# revision 9
# speedup vs baseline: 1.0706x; 1.0022x over previous
"""Bass/Tile kernel for one transformer decoder layer, data-parallel over batch.

Per-core work (one batch element): LN1 -> QKV -> causal attention with
query-axis softmax -> LN2 -> FFN(gelu).

Math note: the reference softmaxes over the QUERY axis, i.e. each key-column k
is normalized over queries q >= k:
  out[q,d] = sum_k exp(s[q,k]) * (V[k,d] / Z[k]),   Z[k] = sum_{q>=k} exp(s[q,k])
We compute ST = S^T in [k, q] layout (ST = K @ Q^T / sqrt(E)) so Z is a
free-axis reduction and the AV matmul needs no transposes.

v2 design (cost-model-driven):
  - all matmul operands bf16 (weights pre-cast on host; 16/32-bit mixing is
    not supported by the PE): full rate at any moving width N.
  - LN affine (g,b) folded into the following projection weights on host, so
    device LN is just (x-mu)*rstd and the transpose evict is a plain copy.
  - causal truncation of score matmuls at 128-col granularity; the causal
    diag-block mask is applied by an extra matmul (ident.T @ maskd) appended
    to the same PSUM accumulation group -- no vector-engine mask pass.
  - Z comes for free from the exp via activation accum_out.
  - FFN2 accumulates all 32 f-blocks in one PSUM group (both ff halves'
    hid tiles are kept resident); FFN1 streams 256-wide quarters so it can
    start while LN2 is still finishing.
  - weight DMAs issued from the (otherwise idle) GPSIMD queue; x/consts/out
    on the sync queue.
  - evictions split between Act and DVE to balance engine load.
"""

from contextlib import ExitStack

import numpy as np

import concourse.bass as bass
import concourse.tile as tile
import concourse.bacc as bacc
from concourse import mybir

F32 = mybir.dt.float32
BF16 = mybir.dt.bfloat16
AF = mybir.ActivationFunctionType
ALU = mybir.AluOpType

P = 128
T = 1024
E = 1024
H = 16
HD = 64
FF = 4096
NT = T // P
NE = E // P
NF = FF // P
SCALE = 1.0 / 32.0  # 1/sqrt(E)
EPS = 1e-5

# packed f32 const columns
_CONST_COLS = {
    "bqt": (0, NE), "bkt": (NE, NE), "b2t": (2 * NE, NE), "b1t": (3 * NE, NF),
}
_CONST_W = 3 * NE + NF
# packed bf16 const columns
_CONSTB_COLS = {"ident": (0, P), "maskd": (P, P), "bvb": (2 * P, E)}
_CONSTB_W = 2 * P + E


def build_decoder(debug=False, reps=1):
    nc = bacc.Bacc(None, target_bir_lowering=False, debug=debug)

    # ---------------- DRAM I/O ----------------
    x_d = nc.dram_tensor("x", (T, E), F32, kind="ExternalInput")
    wq_d = nc.dram_tensor("wq", (NE, P, E), BF16, kind="ExternalInput")
    wk_d = nc.dram_tensor("wk", (NE, P, E), BF16, kind="ExternalInput")
    wv_d = nc.dram_tensor("wv", (E, E), BF16, kind="ExternalInput")
    w1_d = nc.dram_tensor("w1r", (NF, P, E), BF16, kind="ExternalInput")
    w2_d = nc.dram_tensor("w2r", (NE, P, FF), BF16, kind="ExternalInput")
    c_d = nc.dram_tensor("consts", (P, _CONST_W), F32, kind="ExternalInput")
    cb_d = nc.dram_tensor("constsb", (P, _CONSTB_W), BF16, kind="ExternalInput")
    y_d = nc.dram_tensor("yT", (E, T), BF16, kind="ExternalOutput")

    with ExitStack() as es:
        tc = es.enter_context(tile.TileContext(nc))

        const = es.enter_context(tc.tile_pool(name="const", bufs=1, side="left"))
        psq = es.enter_context(tc.tile_pool(name="psq", bufs=1, space="PSUM"))

        cz = const.tile([P, _CONST_W], F32)
        nc.sync.dma_start(cz, c_d[:, :])
        czb = const.tile([P, _CONSTB_W], BF16)
        nc.sync.dma_start(czb, cb_d[:, :])

        def cview(name):
            o, w = _CONST_COLS[name]
            return cz[:, o:o + w]

        def cbview(name):
            o, w = _CONSTB_COLS[name]
            return czb[:, o:o + w]

        bq_t, bk_t, b2_t, b1_t = (cview("bqt"), cview("bkt"), cview("b2t"),
                                  cview("b1t"))
        identb, maskd, bv_b = cbview("ident"), cbview("maskd"), cbview("bvb")
        eps_t = const.tile([P, 1], F32)
        nc.vector.memset(eps_t, EPS)
        # touch every activation function once so the act-table loads happen
        # during the initial DMA wait, not on the LN/attention critical path
        warm = const.tile([P, 1], F32)
        for fn in (AF.Sqrt, AF.Exp, AF.Gelu, AF.Identity):
            nc.scalar.activation(out=warm, in_=eps_t, func=fn)

        def sc_tile(name):
            return psq.tile([P, 2 * 512], F32, tag="sc", bufs=3, name=name)

        def av_tile(name, dt=F32):
            return psq.tile([P, 512], dt, tag="avtr", bufs=2, name=name)

        rep_cm = tc.For_i(0, reps, 1) if reps > 1 else None
        if rep_cm is not None:
            rep_cm.__enter__()

        # =========== Phase 1: x DMA + LN1 + transpose + V proj, per tile =====
        xnT_pool = tc.alloc_tile_pool(name="xnT", bufs=1, side="right")
        xnT = [xnT_pool.tile([P, T], BF16, tag=f"xnT{j}", name=f"xnT{j}")
               for j in range(NE)]
        concat_pool = tc.alloc_tile_pool(name="concat", bufs=1, side="left")
        concat = concat_pool.tile([P, NT * E], BF16)
        concat_v = concat.rearrange("p (a h d) -> p a h d", a=NT, h=H)
        vt_pool = tc.alloc_tile_pool(name="vtp", bufs=1, side="left")
        vt = [vt_pool.tile([P, E], BF16, tag=f"vt{i}", name=f"vt{i}")
              for i in range(NT)]

        def ln_tile(src_fn, dst_tiles, ti, ln_pool, evict_sel):
            """LN (no affine) of one [P, E] token tile + transpose into
            dst_tiles[j][:, ti*P:(ti+1)*P]."""
            stats = ln_pool.tile([P, 2, 6], F32, tag="bnstats")
            for sg in range(2):
                nc.vector.bn_stats(out=stats[:, sg, :],
                                   in_=src_fn(ti)[:, sg * 512:(sg + 1) * 512])
            mv = ln_pool.tile([P, 2], F32, tag="bnmv")
            nc.vector.bn_aggr(out=mv, in_=stats)
            nc.scalar.activation(out=mv[:, 1:2], in_=mv[:, 1:2], func=AF.Sqrt,
                                 bias=eps_t)
            nc.vector.reciprocal(mv[:, 1:2], mv[:, 1:2])
            xn = ln_pool.tile([P, E], BF16, tag="xn")
            nc.vector.tensor_scalar(
                out=xn, in0=src_fn(ti), scalar1=mv[:, 0:1], scalar2=mv[:, 1:2],
                op0=ALU.subtract, op1=ALU.mult)
            for j in range(NE):
                ptr = av_tile(f"tr{ti}_{j}", dt=BF16)[:, :P]
                nc.tensor.transpose(ptr, xn[:, j * P:(j + 1) * P], identb)
                dst = dst_tiles[j][:, ti * P:(ti + 1) * P]
                nc.scalar.activation(out=dst, in_=ptr, func=AF.Identity)

        with tc.tile_pool(name="ph1", bufs=3, side="left") as ph1, \
             tc.tile_pool(name="xin", bufs=NT, side="left") as xin, \
             tc.tile_pool(name="wv", bufs=NE, side="left") as wvp:
            x_tiles = []
            for ti in range(NT):
                xt = xin.tile([P, E], F32, tag="x")
                for hh in range(2):
                    nc.sync.dma_start(xt[:, hh * 512:(hh + 1) * 512],
                                      x_d[ti * P:(ti + 1) * P,
                                          hh * 512:(hh + 1) * 512])
                x_tiles.append(xt)
            wtl = []
            for ko in range(NE):
                wtile = wvp.tile([P, E], BF16, tag="w", name=f"wv{ko}")
                nc.gpsimd.dma_start(wtile, wv_d[ko * P:(ko + 1) * P, :])
                wtl.append(wtile)
            for ti in range(NT):
                ln_tile(lambda t: x_tiles[t][:, :], xnT, ti, ph1, evict_sel=0)
                # V projection for this token tile (needs xnT[:][ti block])
                ps = sc_tile(f"psv{ti}")
                for nh in range(2):
                    for ko in range(NE):
                        nc.tensor.matmul(
                            ps[:, nh * 512:(nh + 1) * 512],
                            lhsT=xnT[ko][:, ti * P:(ti + 1) * P],
                            rhs=wtl[ko][:, nh * 512:(nh + 1) * 512],
                            start=(ko == 0), stop=(ko == NE - 1))
                nc.vector.tensor_tensor(out=vt[ti], in0=ps, in1=bv_b,
                                        op=ALU.add)

        # ====== Phase 2: per-pair {Q/K proj -> scores+exp(+Z) -> AV} ======
        qk_pool = tc.alloc_tile_pool(name="qk", bufs=3, side="left")
        wqk_pool = tc.alloc_tile_pool(name="wqk", bufs=6, side="left")
        e1p = tc.alloc_tile_pool(name="e1", bufs=32, side="left")
        vpp = tc.alloc_tile_pool(name="vp", bufs=16, side="left")
        zsp = tc.alloc_tile_pool(name="zs", bufs=4, side="left")

        def emit_qk(tt):
            """Q/K projections for pair tt; evicted to bf16 in 512-halves so
            the first score matmuls can start after half an evict."""
            qtt = qk_pool.tile([P, T], BF16, tag="qt", name=f"qt{tt}")
            ktt = qk_pool.tile([P, T], BF16, tag="kt", name=f"kt{tt}")
            for dst, w_d, b_t in ((ktt, wk_d, bk_t), (qtt, wq_d, bq_t)):
                wsl = wqk_pool.tile([P, NE, P], BF16, tag="wqk")
                nc.gpsimd.dma_start(
                    wsl, w_d[tt].rearrange("p (a b) -> p a b", a=NE))
                ps = sc_tile(f"psqk{tt}")
                for th in range(2):
                    for ko in range(NE):
                        nc.tensor.matmul(
                            ps[:, th * 512:(th + 1) * 512],
                            lhsT=wsl[:, ko, :],
                            rhs=xnT[ko][:, th * 512:(th + 1) * 512],
                            start=(ko == 0), stop=(ko == NE - 1))
                for th in range(2):
                    nc.vector.tensor_scalar(
                        out=dst[:, th * 512:(th + 1) * 512],
                        in0=ps[:, th * 512:(th + 1) * 512],
                        scalar1=b_t[:, tt:tt + 1], scalar2=None, op0=ALU.add)
            return qtt, ktt

        def emit_vprime(hp, zhs):
            """1/Z and V' tiles for pair hp (exps already done)."""
            out = {}
            for h in (2 * hp, 2 * hp + 1):
                rz = zsp.tile([P, NT], F32, tag="rz")
                nc.vector.reciprocal(rz, zhs[h])
                vps = []
                for ki in range(NT):
                    vp_t = vpp.tile([P, HD], BF16, tag="vp")
                    nc.vector.tensor_scalar_mul(
                        vp_t, vt[ki][:, h * HD:(h + 1) * HD], rz[:, ki:ki + 1])
                    vps.append(vp_t)
                out[h] = (vps, av_tile(f"psav{h}"))
            return out

        def emit_av_chunk(hp, e1s, vinfo, qi):
            for h in (2 * hp, 2 * hp + 1):
                vps, po_ps = vinfo[h]
                for ki in range(qi + 1):
                    nc.tensor.matmul(
                        po_ps[:, qi * HD:(qi + 1) * HD],
                        lhsT=e1s[(h, ki)][:, qi * P:(qi + 1) * P],
                        rhs=vps[ki],
                        start=(ki == 0), stop=(ki == qi))

        def emit_av_flush(hp, vinfo):
            for h in (2 * hp, 2 * hp + 1):
                _, po_ps = vinfo[h]
                nc.vector.tensor_copy(
                    out=concat_v[:, :, h, :],
                    in_=po_ps.rearrange("p (a d) -> p a d", a=NT))

        def emit_scores_av(tt, qtt, ktt, e1s, zhs, prev):
            """Causal-truncated ST blocks; diag mask folded into the PSUM
            accumulation group as ident.T @ maskd; one wide exp per (h, ki)
            with Z accumulated for free. The previous pair's AV matmuls are
            interleaved per-ki to fill the exp-paced PE stalls."""
            hp, e1s_prev, vinfo = prev if prev is not None else (None,) * 3
            for h in (2 * tt, 2 * tt + 1):
                zhs[h] = zsp.tile([P, NT], F32, tag="zh", name=f"zh{h}")
            for ki in range(NT):
                qs = ki * P
                chunks = []
                c0 = qs
                if c0 % 512:
                    nxt = (c0 // 512 + 1) * 512
                    chunks.append((c0, nxt))
                    c0 = nxt
                while c0 < T:
                    chunks.append((c0, c0 + 512))
                    c0 += 512
                for h in (2 * tt, 2 * tt + 1):
                    po = HD * (h % 2)
                    e1ki = e1p.tile([P, T], BF16, tag="e1t", name=f"e1_{h}_{ki}")
                    e1s[(h, ki)] = e1ki
                    sps = sc_tile(f"pss{h}_{ki}")
                    for (a, b) in chunks:
                        has_diag = a == qs
                        nc.tensor.matmul(
                            sps[:, a:b],
                            lhsT=ktt[po:po + HD, ki * P:(ki + 1) * P],
                            rhs=qtt[po:po + HD, a:b],
                            start=True, stop=not has_diag)
                        if has_diag:
                            nc.tensor.matmul(
                                sps[:, qs:qs + P],
                                lhsT=identb, rhs=maskd,
                                start=False, stop=True, skip_group_check=True)
                    nc.scalar.activation(
                        out=e1ki[:, qs:], in_=sps[:, qs:],
                        func=AF.Exp, scale=SCALE,
                        accum_out=zhs[h][:, ki:ki + 1])
                if hp is not None:
                    emit_av_chunk(hp, e1s_prev, vinfo, ki)
            if hp is not None:
                emit_av_flush(hp, vinfo)

        e1s_by_hp = {}
        zhs = {}
        vinfo_prev = None
        for tt in range(H // 2):
            if tt >= 1:
                vinfo_prev = emit_vprime(tt - 1, zhs)
            qtt, ktt = emit_qk(tt)
            e1s_by_hp[tt] = {}
            prev = (tt - 1, e1s_by_hp.pop(tt - 1), vinfo_prev) if tt >= 1 else None
            emit_scores_av(tt, qtt, ktt, e1s_by_hp[tt], zhs, prev)
        # last pair's AV, not interleaved
        hp = H // 2 - 1
        vinfo = emit_vprime(hp, zhs)
        e1s_last = e1s_by_hp.pop(hp)
        for qi in range(NT):
            emit_av_chunk(hp, e1s_last, vinfo, qi)
        emit_av_flush(hp, vinfo)

        zsp.release(); vpp.release(); e1p.release()
        wqk_pool.release(); qk_pool.release()
        vt_pool.release()
        xnT_pool.release()

        # =========== Phase 3: LN2 + transpose ===========
        anT_pool = tc.alloc_tile_pool(name="anT", bufs=1, side="right")
        anT = [anT_pool.tile([P, T], BF16, tag=f"anT{j}", name=f"anT{j}")
               for j in range(NE)]
        with tc.tile_pool(name="ph4", bufs=3, side="left") as ph4:
            for ti in range(NT):
                ln_tile(lambda t: concat[:, t * E:(t + 1) * E], anT, ti, ph4,
                        evict_sel=1)
        concat_pool.release()

        # =========== Phase 4: FFN ===========
        out_pool = tc.alloc_tile_pool(name="outT", bufs=1, side="right")
        outT = [out_pool.tile([P, T], BF16, tag=f"o{j}", name=f"o{j}")
                for j in range(NE)]
        with tc.tile_pool(name="w1s", bufs=4, side="left") as w1s, \
             tc.tile_pool(name="w2s", bufs=2, side="left") as w2s, \
             tc.tile_pool(name="hid", bufs=NF, side="left") as hidp:
            hid = []
            for fo in range(NF):
                w1t = w1s.tile([P, NE, P], BF16, tag="w1")
                nc.gpsimd.dma_start(
                    w1t, w1_d[fo].rearrange("p (a b) -> p a b", a=NE))
                hid_fo = hidp.tile([P, T], BF16, tag="hid", name=f"hid{fo}")
                ps = sc_tile(f"psf{fo}")
                for qt in range(4):
                    for ko in range(NE):
                        nc.tensor.matmul(
                            ps[:, qt * 256:(qt + 1) * 256],
                            lhsT=w1t[:, ko, :],
                            rhs=anT[ko][:, qt * 256:(qt + 1) * 256],
                            start=(ko == 0), stop=(ko == NE - 1))
                    if qt % 2 == 1:
                        th = qt // 2
                        nc.scalar.activation(
                            out=hid_fo[:, th * 512:(th + 1) * 512],
                            in_=ps[:, th * 512:(th + 1) * 512],
                            func=AF.Gelu, bias=b1_t[:, fo:fo + 1])
                hid.append(hid_fo)
            for eo in range(NE):
                w2t = w2s.tile([P, NF, P], BF16, tag="w2")
                nc.gpsimd.dma_start(
                    w2t, w2_d[eo].rearrange("p (a b) -> p a b", a=NF))
                ps = sc_tile(f"pso{eo}")
                for th in range(2):
                    for kl in range(NF):
                        nc.tensor.matmul(
                            ps[:, th * 512:(th + 1) * 512],
                            lhsT=w2t[:, kl, :],
                            rhs=hid[kl][:, th * 512:(th + 1) * 512],
                            start=(kl == 0), stop=(kl == NF - 1))
                    sl = slice(th * 512, (th + 1) * 512)
                    if eo % 2 == 0:
                        nc.vector.tensor_scalar(
                            out=outT[eo][:, sl], in0=ps[:, sl],
                            scalar1=b2_t[:, eo:eo + 1], scalar2=None,
                            op0=ALU.add)
                    else:
                        nc.scalar.activation(
                            out=outT[eo][:, sl], in_=ps[:, sl],
                            func=AF.Identity, bias=b2_t[:, eo:eo + 1])
                    nc.sync.dma_start(y_d[eo * P:(eo + 1) * P, th * 512:(th + 1) * 512],
                                      outT[eo][:, sl])
        out_pool.release()
        anT_pool.release()
        if rep_cm is not None:
            rep_cm.__exit__(None, None, None)

    nc.compile()
    return nc


def host_inputs(core_x, Wq, bq, Wk, bk, Wv, bv, W1, b1, W2, b2, g1, be1, g2, be2):
    """Per-core in_map: LN affines folded into the following projections
    (q = ln_raw @ (g1*Wq) + (be1 @ Wq + bq), where ln_raw = (x-mu)*rstd),
    weights pre-cast to bf16 in the block-transposed layouts the kernel
    expects."""
    import ml_dtypes

    f = np.float32
    bf = ml_dtypes.bfloat16

    g1 = np.asarray(g1, f); be1 = np.asarray(be1, f)
    g2 = np.asarray(g2, f); be2 = np.asarray(be2, f)
    Wq = np.asarray(Wq, f); Wk = np.asarray(Wk, f); Wv = np.asarray(Wv, f)
    W1 = np.asarray(W1, f); W2 = np.asarray(W2, f)

    bq_e = (be1 @ Wq + np.asarray(bq, f)).astype(f)
    bk_e = (be1 @ Wk + np.asarray(bk, f)).astype(f)
    bv_e = (be1 @ Wv + np.asarray(bv, f)).astype(f)
    b1_e = (be2 @ W1 + np.asarray(b1, f)).astype(f)

    Wq_s = Wq * g1[:, None]
    Wk_s = Wk * g1[:, None]
    Wv_s = Wv * g1[:, None]
    W1_s = W1 * g2[:, None]

    consts = np.zeros((P, _CONST_W), f)

    def put(name, arr):
        o, w = _CONST_COLS[name]
        consts[:, o:o + w] = arr

    put("bqt", bq_e.reshape(NE, P).T)
    put("bkt", bk_e.reshape(NE, P).T)
    put("b2t", np.asarray(b2, f).reshape(NE, P).T)
    put("b1t", b1_e.reshape(NF, P).T)

    constsb = np.zeros((P, _CONSTB_W), f)

    def putb(name, arr):
        o, w = _CONSTB_COLS[name]
        constsb[:, o:o + w] = arr

    putb("ident", np.eye(P, dtype=f))
    putb("maskd", np.where(np.triu(np.ones((P, P), bool)), 0.0, -6000.0))
    putb("bvb", np.broadcast_to(bv_e, (P, E)))

    return {
        "x": np.ascontiguousarray(core_x, f),
        "wq": np.ascontiguousarray(
            Wq_s.reshape(NE, P, NE, P).transpose(2, 1, 0, 3)
            .reshape(NE, P, E)).astype(bf),
        "wk": np.ascontiguousarray(
            Wk_s.reshape(NE, P, NE, P).transpose(2, 1, 0, 3)
            .reshape(NE, P, E)).astype(bf),
        "wv": np.ascontiguousarray(Wv_s).astype(bf),
        "w1r": np.ascontiguousarray(
            W1_s.reshape(NE, P, NF, P).transpose(2, 1, 0, 3)
            .reshape(NF, P, E)).astype(bf),
        "w2r": np.ascontiguousarray(
            W2.reshape(NF, P, NE, P).transpose(2, 1, 0, 3)
            .reshape(NE, P, FF)).astype(bf),
        "consts": consts,
        "constsb": constsb.astype(bf),
    }


# ======================================================================
# Harness entry point: full-input kernel with internal batch sharding
# ======================================================================

_NC_CACHE = {}


def _get_nc():
    if "nc" not in _NC_CACHE:
        _NC_CACHE["nc"] = build_decoder()
    return _NC_CACHE["nc"]


def kernel(x, Wq, bq, Wk, bk, Wv, bv, W1, b1, W2, b2, g1, be1, g2, be2):
    """Full-input entry: x [8, 1024, 1024]; returns [8, 1024, 1024] float32.

    Shards batch across the 8 NeuronCores (one element per core), runs the
    Bass decoder kernel SPMD, and gathers/transposes the per-core outputs.
    """
    from concourse.bass_utils import run_bass_kernel_spmd

    x = np.asarray(x, np.float32)
    B = x.shape[0]
    nc = _get_nc()
    args = tuple(np.asarray(a, np.float32) for a in
                 (Wq, bq, Wk, bk, Wv, bv, W1, b1, W2, b2, g1, be1, g2, be2))
    in_maps = [host_inputs(x[c], *args) for c in range(B)]
    res = run_bass_kernel_spmd(nc, in_maps, core_ids=list(range(B)))
    out = np.stack([np.asarray(r["yT"], np.float32).T for r in res.results],
                   axis=0)
    return np.ascontiguousarray(out, np.float32)


# revision 14
# speedup vs baseline: 1.1747x; 1.0972x over previous
"""Bass/Tile kernel for one transformer decoder layer, data-parallel over batch.

Per-core work (one batch element): LN1 -> QKV -> causal attention with
query-axis softmax -> LN2 -> FFN(gelu).

Math note: the reference softmaxes over the QUERY axis, i.e. each key-column k
is normalized over queries q >= k:
  out[q,d] = sum_k exp(s[q,k]) * (V[k,d] / Z[k]),   Z[k] = sum_{q>=k} exp(s[q,k])
We compute ST = S^T in [k, q] layout (ST = K @ Q^T / sqrt(E)) so Z is a
free-axis reduction and the AV matmul needs no transposes.

v2 design (cost-model-driven):
  - all matmul operands bf16 (weights pre-cast on host; 16/32-bit mixing is
    not supported by the PE): full rate at any moving width N.
  - LN affine (g,b) folded into the following projection weights on host, so
    device LN is just (x-mu)*rstd and the transpose evict is a plain copy.
  - causal truncation of score matmuls at 128-col granularity; the causal
    diag-block mask is applied by an extra matmul (ident.T @ maskd) appended
    to the same PSUM accumulation group -- no vector-engine mask pass.
  - Z comes for free from the exp via activation accum_out.
  - FFN2 accumulates all 32 f-blocks in one PSUM group (both ff halves'
    hid tiles are kept resident); FFN1 streams 256-wide quarters so it can
    start while LN2 is still finishing.
  - weight DMAs issued from the (otherwise idle) GPSIMD queue; x/consts/out
    on the sync queue.
  - evictions split between Act and DVE to balance engine load.
"""

from contextlib import ExitStack

import numpy as np

import concourse.bass as bass
import concourse.tile as tile
import concourse.bacc as bacc
from concourse import mybir

F32 = mybir.dt.float32
BF16 = mybir.dt.bfloat16
AF = mybir.ActivationFunctionType
ALU = mybir.AluOpType

P = 128
T = 1024
E = 1024
H = 16
HD = 64
FF = 4096
NT = T // P
NE = E // P
NF = FF // P
SCALE = 1.0 / 32.0  # 1/sqrt(E)
EPS = 1e-5

# packed f32 const columns
_CONST_COLS = {
    "bqt": (0, NE), "bkt": (NE, NE), "b2t": (2 * NE, NE), "b1t": (3 * NE, NF),
}
_CONST_W = 3 * NE + NF
# packed bf16 const columns
_CONSTB_COLS = {"ident": (0, P), "maskd": (P, P), "bvb": (2 * P, E)}
_CONSTB_W = 2 * P + E


def build_decoder(debug=False, reps=1):
    nc = bacc.Bacc(None, target_bir_lowering=False, debug=debug)

    # ---------------- DRAM I/O ----------------
    x_d = nc.dram_tensor("x", (T, E), F32, kind="ExternalInput")
    wq_d = nc.dram_tensor("wq", (NE, P, E), BF16, kind="ExternalInput")
    wk_d = nc.dram_tensor("wk", (NE, P, E), BF16, kind="ExternalInput")
    wv_d = nc.dram_tensor("wv", (E, E), BF16, kind="ExternalInput")
    w1_d = nc.dram_tensor("w1r", (NF, P, E), BF16, kind="ExternalInput")
    w2_d = nc.dram_tensor("w2r", (NE, P, FF), BF16, kind="ExternalInput")
    c_d = nc.dram_tensor("consts", (P, _CONST_W), F32, kind="ExternalInput")
    cb_d = nc.dram_tensor("constsb", (P, _CONSTB_W), BF16, kind="ExternalInput")
    y_d = nc.dram_tensor("yT", (E, T), BF16, kind="ExternalOutput")

    with ExitStack() as es:
        tc = es.enter_context(tile.TileContext(nc))

        const = es.enter_context(tc.tile_pool(name="const", bufs=1, side="left"))
        psq = es.enter_context(tc.tile_pool(name="psq", bufs=1, space="PSUM"))

        cz = const.tile([P, _CONST_W], F32)
        nc.sync.dma_start(cz, c_d[:, :])
        czb = const.tile([P, _CONSTB_W], BF16)
        nc.sync.dma_start(czb, cb_d[:, :])

        def cview(name):
            o, w = _CONST_COLS[name]
            return cz[:, o:o + w]

        def cbview(name):
            o, w = _CONSTB_COLS[name]
            return czb[:, o:o + w]

        bq_t, bk_t, b2_t, b1_t = (cview("bqt"), cview("bkt"), cview("b2t"),
                                  cview("b1t"))
        identb, maskd, bv_b = cbview("ident"), cbview("maskd"), cbview("bvb")
        eps_t = const.tile([P, 1], F32)
        nc.vector.memset(eps_t, EPS)
        # touch every activation function once so the act-table loads happen
        # during the initial DMA wait, not on the LN/attention critical path
        warm = const.tile([P, 1], F32)
        for fn in (AF.Exp, AF.Gelu, AF.Sqrt):  # end on Sqrt: first one needed
            nc.scalar.activation(out=warm, in_=eps_t, func=fn)

        def sc_tile(name):
            # 2-bank tiles: V/FFN accumulators and wide score tiles (ki<4)
            return psq.tile([P, 2 * 512], F32, tag="sc2", bufs=2, name=name)

        def sc1_tile(name):
            # 1-bank tiles: QK projection halves and narrow score tiles (ki>=4)
            return psq.tile([P, 512], F32, tag="sc1", bufs=2, name=name)

        def av_tile(name, dt=F32):
            return psq.tile([P, 512], dt, tag="avtr", bufs=2, name=name)

        rep_cm = tc.For_i(0, reps, 1) if reps > 1 else None
        if rep_cm is not None:
            rep_cm.__enter__()

        # =========== Phase 1: x DMA + LN1 + transpose + V proj, per tile =====
        xnT_pool = tc.alloc_tile_pool(name="xnT", bufs=1, side="right")
        xnT = [xnT_pool.tile([P, T], BF16, tag=f"xnT{j}", name=f"xnT{j}")
               for j in range(NE)]
        concat_pool = tc.alloc_tile_pool(name="concat", bufs=1, side="left")
        concat = concat_pool.tile([P, NT * E], BF16)
        concat_v = concat.rearrange("p (a h d) -> p a h d", a=NT, h=H)
        vt_pool = tc.alloc_tile_pool(name="vtp", bufs=1, side="left")
        vt = [vt_pool.tile([P, E], BF16, tag=f"vt{i}", name=f"vt{i}")
              for i in range(NT)]

        def ln_tile(src_fn, dst_tiles, ti, ln_pool, evict_sel):
            """LN (no affine) of one [P, E] token tile + transpose into
            dst_tiles[j][:, ti*P:(ti+1)*P]."""
            stats = ln_pool.tile([P, 2, 6], F32, tag="bnstats")
            for sg in range(2):
                nc.vector.bn_stats(out=stats[:, sg, :],
                                   in_=src_fn(ti)[:, sg * 512:(sg + 1) * 512])
            mv = ln_pool.tile([P, 2], F32, tag="bnmv")
            nc.vector.bn_aggr(out=mv, in_=stats)
            nc.scalar.activation(out=mv[:, 1:2], in_=mv[:, 1:2], func=AF.Sqrt,
                                 bias=eps_t)
            nc.vector.reciprocal(mv[:, 1:2], mv[:, 1:2])
            xn = ln_pool.tile([P, E], BF16, tag="xn")
            nc.vector.tensor_scalar(
                out=xn, in0=src_fn(ti), scalar1=mv[:, 0:1], scalar2=mv[:, 1:2],
                op0=ALU.subtract, op1=ALU.mult)
            for j in range(NE):
                ptr = av_tile(f"tr{ti}_{j}", dt=BF16)[:, :P]
                nc.tensor.transpose(ptr, xn[:, j * P:(j + 1) * P], identb)
                dst = dst_tiles[j][:, ti * P:(ti + 1) * P]
                nc.scalar.activation(out=dst, in_=ptr, func=AF.Identity)

        with tc.tile_pool(name="ph1", bufs=3, side="left") as ph1, \
             tc.tile_pool(name="xin", bufs=NT, side="left") as xin, \
             tc.tile_pool(name="wv", bufs=NE, side="left") as wvp:
            x_tiles = []
            for ti in range(NT):
                xt = xin.tile([P, E], F32, tag="x")
                for hh in range(2):
                    nc.sync.dma_start(xt[:, hh * 512:(hh + 1) * 512],
                                      x_d[ti * P:(ti + 1) * P,
                                          hh * 512:(hh + 1) * 512])
                x_tiles.append(xt)
            wtl = []
            for ko in range(NE):
                wtile = wvp.tile([P, E], BF16, tag="w", name=f"wv{ko}")
                nc.gpsimd.dma_start(wtile, wv_d[ko * P:(ko + 1) * P, :])
                wtl.append(wtile)
            for ti in range(NT):
                ln_tile(lambda t: x_tiles[t][:, :], xnT, ti, ph1, evict_sel=0)
                # V projection for this token tile (needs xnT[:][ti block])
                ps = sc_tile(f"psv{ti}")
                for nh in range(2):
                    for ko in range(NE):
                        nc.tensor.matmul(
                            ps[:, nh * 512:(nh + 1) * 512],
                            lhsT=xnT[ko][:, ti * P:(ti + 1) * P],
                            rhs=wtl[ko][:, nh * 512:(nh + 1) * 512],
                            start=(ko == 0), stop=(ko == NE - 1))
                nc.vector.tensor_tensor(out=vt[ti], in0=ps, in1=bv_b,
                                        op=ALU.add)

        # ====== Phase 2: per-pair {Q/K proj -> scores+exp(+Z) -> AV} ======
        qk_pool = tc.alloc_tile_pool(name="qk", bufs=2, side="left")
        wqk_pool = tc.alloc_tile_pool(name="wqk", bufs=6, side="left")
        e1p = tc.alloc_tile_pool(name="e1", bufs=32, side="left")
        vpp = tc.alloc_tile_pool(name="vp", bufs=16, side="left")
        zsp = tc.alloc_tile_pool(name="zs", bufs=4, side="left")

        def emit_qk_alloc(tt):
            qtt = qk_pool.tile([P, T], BF16, tag="qt", name=f"qt{tt}")
            ktt = qk_pool.tile([P, T], BF16, tag="kt", name=f"kt{tt}")
            wsl_k = wqk_pool.tile([P, NE, P], BF16, tag="wqk")
            nc.gpsimd.dma_start(
                wsl_k, wk_d[tt].rearrange("p (a b) -> p a b", a=NE))
            wsl_q = wqk_pool.tile([P, NE, P], BF16, tag="wqk")
            nc.gpsimd.dma_start(
                wsl_q, wq_d[tt].rearrange("p (a b) -> p a b", a=NE))
            return {"q": (qtt, wsl_q, bq_t), "k": (ktt, wsl_k, bk_t), "tt": tt}

        # group order: K half0, Q half0, Q half1, K half1
        _QK_GROUPS = (("k", 0), ("q", 0), ("q", 1), ("k", 1))

        def emit_qk_group(qk, gi):
            which, th = _QK_GROUPS[gi]
            dst, wsl, b_t = qk[which]
            tt = qk["tt"]
            ps = sc1_tile(f"psqk{tt}_{gi}")
            for ko in range(NE):
                nc.tensor.matmul(
                    ps, lhsT=wsl[:, ko, :],
                    rhs=xnT[ko][:, th * 512:(th + 1) * 512],
                    start=(ko == 0), stop=(ko == NE - 1))
            nc.vector.tensor_scalar(
                out=dst[:, th * 512:(th + 1) * 512], in0=ps,
                scalar1=b_t[:, tt:tt + 1], scalar2=None, op0=ALU.add)

        def emit_vprime(hp, zhs):
            """1/Z and V' tiles for pair hp (exps already done)."""
            out = {}
            for h in (2 * hp, 2 * hp + 1):
                rz = zsp.tile([P, NT], F32, tag="rz")
                nc.vector.reciprocal(rz, zhs[h])
                vps = []
                for ki in range(NT):
                    vp_t = vpp.tile([P, HD], BF16, tag="vp")
                    nc.vector.tensor_scalar_mul(
                        vp_t, vt[ki][:, h * HD:(h + 1) * HD], rz[:, ki:ki + 1])
                    vps.append(vp_t)
                out[h] = (vps, av_tile(f"psav{h}"))
            return out

        def emit_av_chunk(hp, e1s, vinfo, qi):
            for h in (2 * hp, 2 * hp + 1):
                vps, po_ps = vinfo[h]
                for ki in range(qi + 1):
                    nc.tensor.matmul(
                        po_ps[:, qi * HD:(qi + 1) * HD],
                        lhsT=e1s[(h, ki)][:, qi * P:(qi + 1) * P],
                        rhs=vps[ki],
                        start=(ki == 0), stop=(ki == qi))

        def emit_av_flush(hp, vinfo):
            for h in (2 * hp, 2 * hp + 1):
                _, po_ps = vinfo[h]
                nc.vector.tensor_copy(
                    out=concat_v[:, :, h, :],
                    in_=po_ps.rearrange("p (a d) -> p a d", a=NT))

        def emit_scores_av(tt, qk, e1s, zhs, prev, qk_next):
            """Causal-truncated ST blocks; diag mask folded into the PSUM
            accumulation group as ident.T @ maskd; one wide exp per (h, ki)
            with Z accumulated for free. The previous pair's AV matmuls and
            the NEXT pair's QK projection groups are interleaved per-ki to
            fill the exp-paced PE stalls (they don't depend on this pair's
            activations)."""
            qtt, ktt = qk["q"][0], qk["k"][0]
            hp, e1s_prev, vinfo = prev if prev is not None else (None,) * 3
            for h in (2 * tt, 2 * tt + 1):
                zhs[h] = zsp.tile([P, NT], F32, tag="zh", name=f"zh{h}")
            for ki in range(NT):
                qs = ki * P
                chunks = []
                c0 = qs
                if c0 % 512:
                    nxt = (c0 // 512 + 1) * 512
                    chunks.append((c0, nxt))
                    c0 = nxt
                while c0 < T:
                    chunks.append((c0, c0 + 512))
                    c0 += 512
                for h in (2 * tt, 2 * tt + 1):
                    po = HD * (h % 2)
                    e1ki = e1p.tile([P, T], BF16, tag="e1t", name=f"e1_{h}_{ki}")
                    e1s[(h, ki)] = e1ki
                    if ki < 4:
                        sps = sc_tile(f"pss{h}_{ki}")
                        off = 0
                    else:
                        sps = sc1_tile(f"pss{h}_{ki}")
                        off = 512
                    for (a, b) in chunks:
                        has_diag = a == qs
                        nc.tensor.matmul(
                            sps[:, a - off:b - off],
                            lhsT=ktt[po:po + HD, ki * P:(ki + 1) * P],
                            rhs=qtt[po:po + HD, a:b],
                            start=True, stop=not has_diag)
                        if has_diag:
                            nc.tensor.matmul(
                                sps[:, qs - off:qs - off + P],
                                lhsT=identb, rhs=maskd,
                                start=False, stop=True, skip_group_check=True)
                    nc.scalar.activation(
                        out=e1ki[:, qs:], in_=sps[:, qs - off:],
                        func=AF.Exp, scale=SCALE,
                        accum_out=zhs[h][:, ki:ki + 1])
                if qk_next is not None and ki % 2 == 1:
                    emit_qk_group(qk_next, ki // 2)
                if hp is not None:
                    emit_av_chunk(hp, e1s_prev, vinfo, ki)
            if hp is not None:
                emit_av_flush(hp, vinfo)

        e1s_by_hp = {}
        zhs = {}
        vinfo_prev = None
        qk_cur = emit_qk_alloc(0)
        for gi in range(4):
            emit_qk_group(qk_cur, gi)
        for tt in range(H // 2):
            if tt >= 1:
                vinfo_prev = emit_vprime(tt - 1, zhs)
            qk_next = emit_qk_alloc(tt + 1) if tt + 1 < H // 2 else None
            e1s_by_hp[tt] = {}
            prev = (tt - 1, e1s_by_hp.pop(tt - 1), vinfo_prev) if tt >= 1 else None
            emit_scores_av(tt, qk_cur, e1s_by_hp[tt], zhs, prev, qk_next)
            qk_cur = qk_next
        # last pair's AV, not interleaved
        hp = H // 2 - 1
        vinfo = emit_vprime(hp, zhs)
        e1s_last = e1s_by_hp.pop(hp)
        for qi in range(NT):
            emit_av_chunk(hp, e1s_last, vinfo, qi)
        emit_av_flush(hp, vinfo)

        zsp.release(); vpp.release(); e1p.release()
        wqk_pool.release(); qk_pool.release()
        vt_pool.release()
        xnT_pool.release()

        # =========== Phase 3: LN2 + transpose ===========
        anT_pool = tc.alloc_tile_pool(name="anT", bufs=1, side="right")
        anT = [anT_pool.tile([P, T], BF16, tag=f"anT{j}", name=f"anT{j}")
               for j in range(NE)]
        with tc.tile_pool(name="ph4", bufs=3, side="left") as ph4:
            for ti in range(NT):
                ln_tile(lambda t: concat[:, t * E:(t + 1) * E], anT, ti, ph4,
                        evict_sel=1)
        concat_pool.release()

        # =========== Phase 4: FFN ===========
        out_pool = tc.alloc_tile_pool(name="outT", bufs=1, side="right")
        outT = [out_pool.tile([P, T], BF16, tag=f"o{j}", name=f"o{j}")
                for j in range(NE)]
        with tc.tile_pool(name="w1s", bufs=4, side="left") as w1s, \
             tc.tile_pool(name="w2s", bufs=2, side="left") as w2s, \
             tc.tile_pool(name="hid", bufs=NF, side="left") as hidp:
            hid = []
            for fo in range(NF):
                w1t = w1s.tile([P, NE, P], BF16, tag="w1")
                nc.gpsimd.dma_start(
                    w1t, w1_d[fo].rearrange("p (a b) -> p a b", a=NE))
                hid_fo = hidp.tile([P, T], BF16, tag="hid", name=f"hid{fo}")
                ps = sc_tile(f"psf{fo}")
                for qt in range(4):
                    for ko in range(NE):
                        nc.tensor.matmul(
                            ps[:, qt * 256:(qt + 1) * 256],
                            lhsT=w1t[:, ko, :],
                            rhs=anT[ko][:, qt * 256:(qt + 1) * 256],
                            start=(ko == 0), stop=(ko == NE - 1))
                    if qt % 2 == 1:
                        th = qt // 2
                        nc.scalar.activation(
                            out=hid_fo[:, th * 512:(th + 1) * 512],
                            in_=ps[:, th * 512:(th + 1) * 512],
                            func=AF.Gelu, bias=b1_t[:, fo:fo + 1])
                hid.append(hid_fo)
            for eo in range(NE):
                w2t = w2s.tile([P, NF, P], BF16, tag="w2")
                nc.gpsimd.dma_start(
                    w2t, w2_d[eo].rearrange("p (a b) -> p a b", a=NF))
                ps = sc_tile(f"pso{eo}")
                for th in range(2):
                    for kl in range(NF):
                        nc.tensor.matmul(
                            ps[:, th * 512:(th + 1) * 512],
                            lhsT=w2t[:, kl, :],
                            rhs=hid[kl][:, th * 512:(th + 1) * 512],
                            start=(kl == 0), stop=(kl == NF - 1))
                    sl = slice(th * 512, (th + 1) * 512)
                    if eo % 2 == 0:
                        nc.vector.tensor_scalar(
                            out=outT[eo][:, sl], in0=ps[:, sl],
                            scalar1=b2_t[:, eo:eo + 1], scalar2=None,
                            op0=ALU.add)
                    else:
                        nc.scalar.activation(
                            out=outT[eo][:, sl], in_=ps[:, sl],
                            func=AF.Identity, bias=b2_t[:, eo:eo + 1])
                    nc.sync.dma_start(y_d[eo * P:(eo + 1) * P, th * 512:(th + 1) * 512],
                                      outT[eo][:, sl])
        out_pool.release()
        anT_pool.release()
        if rep_cm is not None:
            rep_cm.__exit__(None, None, None)

    nc.compile()
    return nc


def host_inputs(core_x, Wq, bq, Wk, bk, Wv, bv, W1, b1, W2, b2, g1, be1, g2, be2):
    """Per-core in_map: LN affines folded into the following projections
    (q = ln_raw @ (g1*Wq) + (be1 @ Wq + bq), where ln_raw = (x-mu)*rstd),
    weights pre-cast to bf16 in the block-transposed layouts the kernel
    expects."""
    import ml_dtypes

    f = np.float32
    bf = ml_dtypes.bfloat16

    g1 = np.asarray(g1, f); be1 = np.asarray(be1, f)
    g2 = np.asarray(g2, f); be2 = np.asarray(be2, f)
    Wq = np.asarray(Wq, f); Wk = np.asarray(Wk, f); Wv = np.asarray(Wv, f)
    W1 = np.asarray(W1, f); W2 = np.asarray(W2, f)

    bq_e = (be1 @ Wq + np.asarray(bq, f)).astype(f)
    bk_e = (be1 @ Wk + np.asarray(bk, f)).astype(f)
    bv_e = (be1 @ Wv + np.asarray(bv, f)).astype(f)
    b1_e = (be2 @ W1 + np.asarray(b1, f)).astype(f)

    Wq_s = Wq * g1[:, None]
    Wk_s = Wk * g1[:, None]
    Wv_s = Wv * g1[:, None]
    W1_s = W1 * g2[:, None]

    consts = np.zeros((P, _CONST_W), f)

    def put(name, arr):
        o, w = _CONST_COLS[name]
        consts[:, o:o + w] = arr

    put("bqt", bq_e.reshape(NE, P).T)
    put("bkt", bk_e.reshape(NE, P).T)
    put("b2t", np.asarray(b2, f).reshape(NE, P).T)
    put("b1t", b1_e.reshape(NF, P).T)

    constsb = np.zeros((P, _CONSTB_W), f)

    def putb(name, arr):
        o, w = _CONSTB_COLS[name]
        constsb[:, o:o + w] = arr

    putb("ident", np.eye(P, dtype=f))
    putb("maskd", np.where(np.triu(np.ones((P, P), bool)), 0.0, -6000.0))
    putb("bvb", np.broadcast_to(bv_e, (P, E)))

    return {
        "x": np.ascontiguousarray(core_x, f),
        "wq": np.ascontiguousarray(
            Wq_s.reshape(NE, P, NE, P).transpose(2, 1, 0, 3)
            .reshape(NE, P, E)).astype(bf),
        "wk": np.ascontiguousarray(
            Wk_s.reshape(NE, P, NE, P).transpose(2, 1, 0, 3)
            .reshape(NE, P, E)).astype(bf),
        "wv": np.ascontiguousarray(Wv_s).astype(bf),
        "w1r": np.ascontiguousarray(
            W1_s.reshape(NE, P, NF, P).transpose(2, 1, 0, 3)
            .reshape(NF, P, E)).astype(bf),
        "w2r": np.ascontiguousarray(
            W2.reshape(NF, P, NE, P).transpose(2, 1, 0, 3)
            .reshape(NE, P, FF)).astype(bf),
        "consts": consts,
        "constsb": constsb.astype(bf),
    }


# ======================================================================
# Harness entry point: full-input kernel with internal batch sharding
# ======================================================================

_NC_CACHE = {}


def _get_nc():
    if "nc" not in _NC_CACHE:
        _NC_CACHE["nc"] = build_decoder()
    return _NC_CACHE["nc"]


def kernel(x, Wq, bq, Wk, bk, Wv, bv, W1, b1, W2, b2, g1, be1, g2, be2):
    """Full-input entry: x [8, 1024, 1024]; returns [8, 1024, 1024] float32.

    Shards batch across the 8 NeuronCores (one element per core), runs the
    Bass decoder kernel SPMD, and gathers/transposes the per-core outputs.
    """
    from concourse.bass_utils import run_bass_kernel_spmd

    x = np.asarray(x, np.float32)
    B = x.shape[0]
    nc = _get_nc()
    args = tuple(np.asarray(a, np.float32) for a in
                 (Wq, bq, Wk, bk, Wv, bv, W1, b1, W2, b2, g1, be1, g2, be2))
    in_maps = [host_inputs(x[c], *args) for c in range(B)]
    res = run_bass_kernel_spmd(nc, in_maps, core_ids=list(range(B)))
    out = np.stack([np.asarray(r["yT"], np.float32).T for r in res.results],
                   axis=0)
    return np.ascontiguousarray(out, np.float32)
